# revision 1
# baseline (speedup 1.0000x reference)
"""Trainium2 Bass kernel for nn_DecoderBlock (self-attn + cross-attn + FFN, post-LN).

Sharding: 8 cores = (batch b in {0,1}) x (query block qi in {0..3} of 512 rows).
Each core computes its 512 output rows end-to-end; K/V work over the full
sequence is replicated inside a batch (no cross-core communication).

All on-chip activations are kept transposed [d, s] so every GEMM consumes
natural weight layouts and no on-device transposes are needed. The host
transposes x/enc on the way in and the output on the way out.

Attention uses the transposed layout: S^T[k,q] = K_h^T(dk,k)^T-free matmuls
with two heads packed into the 128-row PE array via tile_position; softmax is
exp(s/8 - 4) with the normalizer produced by an extra ones-column on V
(M=65 matmul) and divided out after accumulation. Causal masking is applied
as a per-core 0/1 mask multiply on the exp tiles (mask content differs per
core; the program is identical across cores). The cross-attention key mask
(src_mask) is folded into the V rows instead.

All matmuls run in float32r (measured ~1.5e-4 rel err vs fp32).
"""

import numpy as np

import concourse.bass as bass
import concourse.mybir as mybir
import concourse.tile as tile
from concourse import bacc
from concourse.bass import ds
from concourse.bass_utils import run_bass_kernel_spmd

F32 = mybir.dt.float32
F32R = mybir.dt.float32r
AF = mybir.ActivationFunctionType
ALU = mybir.AluOpType

B, S, D, H, DK, DFF = 2, 2048, 1024, 16, 64, 4096
NCORES = 8
QS = 512            # query rows per core
DC = D // 128       # 8 d-chunks
FC = DFF // 128     # 32 dff-chunks
PANEL = 512         # kpos panel size
NPANEL = S // PANEL # 4
NSC = PANEL // 128  # 4 kpos chunks per panel
NHP = H // 2        # 8 head pairs
LN_EPS = 1e-5
EXP_BIAS = -4.0     # exp(s/8 - 4): overflow safety; cancels in the normalizer


def _dchunks(ap):
    """[D-like, N] dram AP -> [128, chunks, N] (partition = row % 128)."""
    return ap.rearrange("(c p) n -> p c n", p=128)


def _wpairs(ap):
    """[K, M] weight AP -> [128, K//128, M]; slice pairs of K-chunks."""
    return ap.rearrange("(c p) m -> p c m", p=128)


tap_layout = {}


def _build(tap=None):
    nc = bacc.Bacc("TRN2", target_bir_lowering=False, debug=False,
                   num_devices=NCORES)

    def inp(name, shape):
        return nc.dram_tensor(name, shape, F32, kind="ExternalInput").ap()

    xoT = inp("xoT", [D, QS])          # x[b].T[:, q0:q0+QS]
    xT = inp("xT", [D, S])             # x[b].T
    eT = inp("eT", [D, S])             # enc[b].T
    msk = inp("msk", [S // 128, 128, QS])  # per-core causal mask (k-chunk, k, q)
    vms = inp("vms", [128, S // 128])  # src_mask per kpos, per-partition layout
    w_sa = {t: inp(f"w_sa{t}", [D, D]) for t in "qkvo"}
    w_ca = {t: inp(f"w_ca{t}", [D, D]) for t in "qkvo"}
    w_ff1 = inp("w_ff1", [D, DFF])
    w_ff2 = inp("w_ff2", [DFF, D])
    fb1 = inp("fb1", [128, FC])        # ff_b1 in [128, chunk] layout
    fb2 = inp("fb2", [128, DC])
    lnb = inp("lnb", [128, 6 * DC])    # g1,b1,g2,b2,g3,b3 packed
    outT = nc.dram_tensor("outT", [128, DC, QS], F32, kind="ExternalOutput").ap()
    dbg = nc.dram_tensor("dbg", [128, 40, QS], F32, kind="ExternalOutput").ap() \
        if tap else None
    tapped = []

    def tapit(name, ap):
        if tap and (tap == "all" or name in tap):
            tapped.append((name, ap))

    with tile.TileContext(nc) as tc:
        with tc.tile_pool(name="glob", bufs=1) as G, \
             tc.tile_pool(name="acts", bufs=2) as ACTS, \
             tc.tile_pool(name="ps2", bufs=2, space="PSUM") as PS2, \
             tc.tile_pool(name="ps1", bufs=1, space="PSUM") as PS1:

            ones_f = G.tile([128, 64], F32)
            nc.vector.memset(ones_f[:], 1.0)
            ones = G.tile([128, 1], F32R)
            nc.vector.tensor_copy(ones[:], ones_f[:, 0:1])
            cexpb = G.tile([128, 1], F32)
            nc.vector.memset(cexpb[:], EXP_BIAS)
            cleps = G.tile([128, 1], F32)
            nc.vector.memset(cleps[:], LN_EPS)
            lnbt = G.tile([128, 6 * DC], F32)
            nc.sync.dma_start(lnbt[:], lnb)
            fb1t = G.tile([128, FC], F32)
            nc.sync.dma_start(fb1t[:], fb1)
            fb2t = G.tile([128, DC], F32)
            nc.sync.dma_start(fb2t[:], fb2)
            vmst = G.tile([128, S // 128], F32)
            nc.sync.dma_start(vmst[:], vms)
            stats = G.tile([1, 8, QS], F32)

            def proj_from_dram(wdram, rhs, evict, n_mc=DC, n_kc=DC, wtag="w"):
                """psum[mc] = sum_kc w[kc,mc-chunk].T @ rhs[:,kc,:]; evict(mc, psum)."""
                wre = _wpairs(wdram)
                for mc in range(n_mc):
                    ps = PS2.tile([128, QS], F32, tag="pj")
                    for k2 in range(n_kc // 2):
                        wt = WPOOL.tile([128, 2, 128], F32R, tag=wtag)
                        nc.sync.dma_start(
                            wt[:],
                            wre[:, 2 * k2:2 * k2 + 2, ds(mc * 128, 128)].bitcast(F32R))
                        for j in range(2):
                            kc = 2 * k2 + j
                            nc.tensor.matmul(ps[:], wt[:, j, :], rhs[:, kc, :],
                                             start=(kc == 0), stop=(kc == n_kc - 1))
                    evict(mc, ps)

            def layernorm(xpre, gcol, bcol, out, TMP):
                """out[:,mc,:] = (xpre - mu)/sd * g + b, stats over d (partition+chunks)."""
                pmu = PS2.tile([1, QS], F32, tag="pj")
                for kc in range(DC):
                    nc.tensor.matmul(pmu[:], ones[:], xpre[:, kc, :],
                                     start=(kc == 0), stop=(kc == DC - 1))
                pm2 = PS2.tile([1, QS], F32, tag="pj")
                for kc in range(DC):
                    sq = TMP.tile([128, QS], F32R, tag="sq")
                    nc.scalar.activation(sq[:], xpre[:, kc, :], AF.Square)
                    nc.tensor.matmul(pm2[:], ones[:], sq[:],
                                     start=(kc == 0), stop=(kc == DC - 1))
                mu = stats[0:1, 0, :]
                ex2 = stats[0:1, 1, :]
                var = stats[0:1, 2, :]
                sd = stats[0:1, 3, :]
                rstd = stats[0:1, 4, :]
                nc.vector.tensor_scalar_mul(mu, pmu[:], 1.0 / D)
                nc.vector.tensor_scalar_mul(ex2, pm2[:], 1.0 / D)
                nc.vector.tensor_tensor(var, mu, mu, ALU.mult)
                nc.vector.tensor_sub(var, ex2, var)
                nc.scalar.activation(sd, var, AF.Sqrt, bias=cleps[0:1, :])
                nc.vector.reciprocal(rstd, sd)
                mub = TMP.tile([128, QS], F32, tag="mub")
                nc.gpsimd.partition_broadcast(mub[:], mu)
                rsb = TMP.tile([128, QS], F32, tag="rsb")
                nc.gpsimd.partition_broadcast(rsb[:], rstd)
                for mc in range(DC):
                    t = TMP.tile([128, QS], F32, tag="t")
                    nc.vector.tensor_sub(t[:], xpre[:, mc, :], mub[:])
                    nc.vector.tensor_mul(t[:], t[:], rsb[:])
                    nc.vector.tensor_scalar(
                        out=out[:, mc, :], in0=t[:],
                        scalar1=gcol[:, mc:mc + 1], scalar2=bcol[:, mc:mc + 1],
                        op0=ALU.mult, op1=ALU.add)

            def attention(qsrc, kv_dram, w, res, gcol, bcol, masked):
                nonlocal WPOOL
                sfx = "sa" if masked else "ca"
                kvre = _dchunks(kv_dram)
                with tc.tile_pool(name="attn", bufs=1) as A:
                    QT = A.tile([128, DC, QS], F32R)
                    oacc = A.tile([65, H, QS], F32)
                    with tc.tile_pool(name="wq", bufs=4) as WPOOL:
                        def evq(mc, ps):
                            nc.vector.tensor_copy(QT[:, mc, :], ps[:])
                        proj_from_dram(w["q"], qsrc, evq)
                    tapit("QT" + ("sa" if masked else "ca"), QT)

                    with tc.tile_pool(name="panel", bufs=1) as P, \
                         tc.tile_pool(name="wkp", bufs=4) as WPOOL, \
                         tc.tile_pool(name="wvp", bufs=3) as WV, \
                         tc.tile_pool(name="pp", bufs=2) as PP, \
                         tc.tile_pool(name="xpp", bufs=2) as XPP:
                        for p in range(NPANEL):
                            xp = XPP.tile([128, DC, PANEL], F32R, tag="xp")
                            nc.sync.dma_start(
                                xp[:], kvre[:, :, ds(p * PANEL, PANEL)].bitcast(F32R))
                            KT = P.tile([128, DC, PANEL], F32R, tag="kt")

                            def evk(mc, ps):
                                nc.vector.tensor_copy(KT[:, mc, :], ps[:])
                            proj_from_dram(w["k"], xp, evk)
                            if p == 0:
                                tapit("KT" + ("sa" if masked else "ca"), KT)

                            v1 = P.tile([128, NSC, H, DK + 1], F32R, tag="v1")
                            nc.vector.tensor_copy(
                                v1[:, :, :, DK],
                                ones_f[:].rearrange("p (a b) -> p a b", a=NSC))
                            wvre = _dchunks(w["v"])
                            for nh in range(2):
                                pss = [PS2.tile([128, QS], F32, tag="s0", name="vps0"),
                                       PS2.tile([128, QS], F32, tag="s1", name="vps1"),
                                       PS1.tile([128, QS], F32, tag="o0", name="vps2"),
                                       PS1.tile([128, QS], F32, tag="o1", name="vps3")]
                                for kc in range(DC):
                                    wvt = WV.tile([128, PANEL], F32R, tag="wv")
                                    nc.sync.dma_start(
                                        wvt[:],
                                        wvre[:, kc, ds(nh * 512, 512)].bitcast(F32R))
                                    for sc in range(NSC):
                                        nc.tensor.matmul(
                                            pss[sc][:], xp[:, kc, ds(sc * 128, 128)],
                                            wvt[:], start=(kc == 0), stop=(kc == DC - 1))
                                for sc in range(NSC):
                                    nc.vector.tensor_copy(
                                        v1[:, sc, nh * 8:(nh + 1) * 8, 0:DK],
                                        pss[sc][:].rearrange("p (a b) -> p a b", a=8))
                            if not masked:
                                # fold src_mask into V rows (incl. ones column)
                                for sc in range(NSC):
                                    nc.vector.tensor_scalar_mul(
                                        v1[:, sc, :, :], v1[:, sc, :, :],
                                        vmst[:, p * NSC + sc:p * NSC + sc + 1])
                            if masked:
                                mt = P.tile([128, NSC, QS], F32, tag="mk")
                                nc.sync.dma_start(
                                    mt[:],
                                    msk[ds(p * NSC, NSC)].rearrange("c p q -> p c q"))
                            for hp in range(NHP):
                                po0 = PS1.tile([65, QS], F32, tag="o0")
                                po1 = PS1.tile([65, QS], F32, tag="o1")
                                for sc in range(NSC):
                                    ps0 = PS2.tile([128, QS], F32, tag="s0")
                                    ps1 = PS2.tile([128, QS], F32, tag="s1")
                                    nc.tensor.matmul(
                                        ps0[:], KT[0:64, hp, ds(sc * 128, 128)],
                                        QT[0:64, hp, :], start=True, stop=True)
                                    nc.tensor.matmul(
                                        ps1[:], KT[64:128, hp, ds(sc * 128, 128)],
                                        QT[64:128, hp, :], start=True, stop=True,
                                        tile_position=(64, 0))
                                    p0 = PP.tile([128, QS], F32R, tag="p0")
                                    p1 = PP.tile([128, QS], F32R, tag="p1")
                                    nc.scalar.activation(p0[:], ps0[:], AF.Exp,
                                                         scale=0.125, bias=cexpb[:])
                                    nc.scalar.activation(p1[:], ps1[:], AF.Exp,
                                                         scale=0.125, bias=cexpb[:])
                                    if masked:
                                        nc.vector.tensor_mul(p0[:], p0[:], mt[:, sc, :])
                                        nc.vector.tensor_mul(p1[:], p1[:], mt[:, sc, :])
                                    nc.tensor.matmul(po0[:], v1[:, sc, 2 * hp, :],
                                                     p0[:], start=(sc == 0),
                                                     stop=(sc == NSC - 1))
                                    nc.tensor.matmul(po1[:], v1[:, sc, 2 * hp + 1, :],
                                                     p1[:], start=(sc == 0),
                                                     stop=(sc == NSC - 1))
                                if p == 0:
                                    nc.vector.tensor_copy(oacc[:, 2 * hp, :], po0[:])
                                    nc.vector.tensor_copy(oacc[:, 2 * hp + 1, :], po1[:])
                                else:
                                    nc.vector.tensor_add(oacc[:, 2 * hp, :],
                                                         oacc[:, 2 * hp, :], po0[:])
                                    nc.vector.tensor_add(oacc[:, 2 * hp + 1, :],
                                                         oacc[:, 2 * hp + 1, :], po1[:])

                    with tc.tile_pool(name="aepi", bufs=1) as E, \
                         tc.tile_pool(name="rnbp", bufs=2) as RNB, \
                         tc.tile_pool(name="tmp", bufs=2) as TMP, \
                         tc.tile_pool(name="wo", bufs=4) as WPOOL:
                        tapit("oacc" + sfx, oacc)
                        rn = E.tile([1, H, QS], F32)
                        nc.vector.reciprocal(rn[:], oacc[64:65, :, :])
                        ON = E.tile([128, DC, QS], F32R)
                        for m in range(DC):
                            rnb = RNB.tile([64, 2, QS], F32, tag="rnb")
                            nc.gpsimd.partition_broadcast(rnb[:, 0, :],
                                                          rn[0:1, 2 * m, :])
                            nc.gpsimd.partition_broadcast(rnb[:, 1, :],
                                                          rn[0:1, 2 * m + 1, :])
                            nc.vector.tensor_mul(ON[0:64, m, :],
                                                 oacc[0:64, 2 * m, :], rnb[:, 0, :])
                            nc.vector.tensor_mul(ON[64:128, m, :],
                                                 oacc[0:64, 2 * m + 1, :], rnb[:, 1, :])
                        xpre = E.tile([128, DC, QS], F32R)

                        def evo(mc, ps):
                            nc.vector.tensor_add(xpre[:, mc, :], ps[:], res[:, mc, :])
                        proj_from_dram(w["o"], ON, evo)
                        tapit("ON" + sfx, ON)
                        tapit("xpre" + sfx, xpre)
                        xnext = ACTS.tile([128, DC, QS], F32R, tag="act")
                        layernorm(xpre, gcol, bcol, xnext, TMP)
                        tapit("xn" + sfx, xnext)
                return xnext

            # ---- load own-query activations ----
            xo = ACTS.tile([128, DC, QS], F32R, tag="act")
            nc.sync.dma_start(xo[:], _dchunks(xoT).bitcast(F32R))

            WPOOL = None
            g1, b1 = lnbt[:, 0:DC], lnbt[:, DC:2 * DC]
            g2, b2 = lnbt[:, 2 * DC:3 * DC], lnbt[:, 3 * DC:4 * DC]
            g3, b3 = lnbt[:, 4 * DC:5 * DC], lnbt[:, 5 * DC:6 * DC]

            x1 = attention(xo, xT, w_sa, xo, g1, b1, masked=True)
            x2 = attention(x1, eT, w_ca, x1, g2, b2, masked=False)

            # ---- FFN ----
            with tc.tile_pool(name="ffn", bufs=1) as F, \
                 tc.tile_pool(name="tmp2", bufs=2) as TMP, \
                 tc.tile_pool(name="wf", bufs=4) as WPOOL:
                h1 = F.tile([128, FC, QS], F32R)

                def ev1(fc, ps):
                    nc.scalar.activation(h1[:, fc, :], ps[:], AF.Relu,
                                         bias=fb1t[:, fc:fc + 1])
                proj_from_dram(w_ff1, x2, ev1, n_mc=FC, n_kc=DC)

                tapit("h1a", h1[:, 0:8, :])
                tapit("h1b", h1[:, 8:16, :])
                xpre = F.tile([128, DC, QS], F32R)

                def ev2(mc, ps):
                    nc.vector.scalar_tensor_tensor(
                        out=xpre[:, mc, :], in0=ps[:],
                        scalar=fb2t[:, mc:mc + 1], in1=x2[:, mc, :],
                        op0=ALU.add, op1=ALU.add)
                proj_from_dram(w_ff2, h1, ev2, n_mc=DC, n_kc=FC)

                tapit("xpreff", xpre)
                out = F.tile([128, DC, QS], F32)
                layernorm(xpre, g3, b3, out, TMP)
                tapit("outf", out)
                tc.strict_bb_all_engine_barrier()
                for mc in range(DC):
                    nc.sync.dma_start(outT[:, mc, :], out[:, mc, :])
            if tap:
                base = 0
                tap_layout.clear()
                for name, t in tapped:
                    sh = t.shape
                    nparts = sh[0]
                    assert len(sh) == 3 and sh[2] == QS
                    tap_layout[name] = (base, sh[1], nparts)
                    for cci in range(sh[1]):
                        nc.sync.dma_start(
                            dbg[0:nparts, base + cci, :].bitcast(t.dtype),
                            t[:, cci, :])
                    base += sh[1]
                assert base <= 40

    nc.compile()
    return nc


_NC_CACHE = None


def _get_nc():
    global _NC_CACHE
    if _NC_CACHE is None:
        _NC_CACHE = _build()
    return _NC_CACHE


def _prep_in_maps(x, enc, tgt_mask, src_mask,
                  sa_wq, sa_wk, sa_wv, sa_wo,
                  ca_wq, ca_wk, ca_wv, ca_wo,
                  ff_w1, ff_b1, ff_w2, ff_b2,
                  ln1_g, ln1_b, ln2_g, ln2_b, ln3_g, ln3_b):
    f32 = np.float32

    def c(a):
        return np.ascontiguousarray(np.asarray(a), dtype=f32)

    xTb = [c(np.asarray(x)[b].T) for b in range(B)]          # [1024, 2048]
    eTb = [c(np.asarray(enc)[b].T) for b in range(B)]
    tm = np.asarray(tgt_mask)[0, 0].astype(f32).T            # [k, q]
    sm = np.asarray(src_mask)[0, 0, 0].astype(f32)           # [k]
    vms = c(sm.reshape(S // 128, 128).T)                     # [128, 16]

    def percol(v, nchunks):
        return c(np.asarray(v).reshape(nchunks, 128).T)

    lnb = c(np.concatenate(
        [percol(v, DC) for v in [ln1_g, ln1_b, ln2_g, ln2_b, ln3_g, ln3_b]],
        axis=1))
    fb1 = percol(ff_b1, FC)
    fb2 = percol(ff_b2, DC)
    shared = {
        "vms": vms, "lnb": lnb, "fb1": fb1, "fb2": fb2,
        "w_saq": c(sa_wq), "w_sak": c(sa_wk), "w_sav": c(sa_wv), "w_sao": c(sa_wo),
        "w_caq": c(ca_wq), "w_cak": c(ca_wk), "w_cav": c(ca_wv), "w_cao": c(ca_wo),
        "w_ff1": c(ff_w1), "w_ff2": c(ff_w2),
    }
    in_maps = []
    for core in range(NCORES):
        b, qi = core // 4, core % 4
        q0 = qi * QS
        m = dict(shared)
        m["xT"] = xTb[b]
        m["eT"] = eTb[b]
        m["xoT"] = c(xTb[b][:, q0:q0 + QS])
        m["msk"] = c(tm[:, q0:q0 + QS].reshape(S // 128, 128, QS))
        in_maps.append(m)
    return in_maps


def _gather_out(res):
    out = np.empty((B, S, D), dtype=np.float32)
    for core in range(NCORES):
        b, qi = core // 4, core % 4
        q0 = qi * QS
        arr = res.results[core]["outT"]  # [128, DC, QS]
        out[b, q0:q0 + QS, :] = arr.transpose(1, 0, 2).reshape(D, QS).T
    return out


def kernel(**inputs):
    in_maps = _prep_in_maps(**inputs)
    nc = _get_nc()
    res = run_bass_kernel_spmd(nc, in_maps, core_ids=list(range(NCORES)))
    return _gather_out(res)


def _profiled_run(inputs):
    """Test-only: run with NTFF tracing to get HW exec time."""
    in_maps = _prep_in_maps(**inputs)
    nc = _get_nc()
    return run_bass_kernel_spmd(nc, in_maps, core_ids=list(range(NCORES)),
                                trace=True)



# revision 9
# speedup vs baseline: 1.3519x; 1.3519x over previous
"""Trainium2 Bass kernel for nn_DecoderBlock (self-attn + cross-attn + FFN, post-LN).

Sharding: 8 cores = (batch b in {0,1}) x (query block qi in {0..3} of 512 rows).
Each core computes its 512 output rows end-to-end. K/V projections are
sharded: each core projects only its own 512-position panel of K and V (for
both attentions) and the panels are exchanged with an AllGather across the
4-core replica group of the batch, removing the 4x-replicated K/V projection
compute of the all-local variant.

All matmuls run in bfloat16 (weights are cast host-side; activations are
rounded to bf16 on chip) with fp32 PSUM accumulation; layernorm statistics
are computed on float32r copies so the stats matmuls stay full-rate.

Attention keeps activations transposed [d, s]: scores use KT chunks as the
stationary operand with two heads packed into the 128-row PE array via
tile_position; softmax is exp(s/8 - 4) with the normalizer produced by an
extra ones-column on V (M=65 matmul) and divided out after accumulation
(reciprocal on the scalar engine). The attention inner loop is head-pair
outer / panel inner so the AV accumulation stays in PSUM across the whole
sequence. Causal masking is a per-core 0/1 bf16 mask multiply on the exp
tiles; the cross-attention key mask (src_mask) is folded into the V rows.
"""

import numpy as np
import ml_dtypes

import concourse.bass as bass
import concourse.mybir as mybir
import concourse.tile as tile
from concourse import bacc
from concourse.bass import ds
from concourse.bass_utils import run_bass_kernel_spmd

F32 = mybir.dt.float32
F32R = mybir.dt.float32r
BF16 = mybir.dt.bfloat16
AF = mybir.ActivationFunctionType
ALU = mybir.AluOpType

B, S, D, H, DK, DFF = 2, 2048, 1024, 16, 64, 4096
NCORES = 8
QS = 512            # query rows per core
DC = D // 128       # 8 d-chunks
FC = DFF // 128     # 32 dff-chunks
PANEL = 512         # kpos panel size (= one core's contribution)
NPANEL = S // PANEL # 4
NSC = PANEL // 128  # 4 kpos chunks per panel
NHP = H // 2        # 8 head pairs
LN_EPS = 1e-5
EXP_BIAS = -4.0     # exp(s/8 - 4): overflow safety; cancels in the normalizer
RG = [[0, 1, 2, 3], [4, 5, 6, 7]]  # replica groups (one per batch)


def _dchunks(ap):
    """[D-like, N] dram AP -> [128, chunks, N] (partition = row % 128)."""
    return ap.rearrange("(c p) n -> p c n", p=128)


def _wpairs(ap):
    """[K, M] weight AP -> [128, K//128, M]; slice pairs of K-chunks."""
    return ap.rearrange("(c p) m -> p c m", p=128)


tap_layout = {}


def _build(tap=None):
    nc = bacc.Bacc("TRN2", target_bir_lowering=False, debug=False,
                   num_devices=NCORES)

    def inp(name, shape, dt=BF16):
        return nc.dram_tensor(name, shape, dt, kind="ExternalInput").ap()

    xoT = inp("xoT", [D, QS])          # x[b].T[:, q0:q0+QS]
    eoT = inp("eoT", [D, QS])          # enc[b].T[:, q0:q0+QS]
    msk = inp("msk", [S // 128, 128, QS])  # per-core causal mask (k-chunk, k, q)
    vms = inp("vms", [128, S // 128], F32)  # src_mask per kpos
    w_sa = {t: inp(f"w_sa{t}", [D, D]) for t in "qkvo"}
    w_ca = {t: inp(f"w_ca{t}", [D, D]) for t in "qkvo"}
    w_ff1 = inp("w_ff1", [D, DFF])
    w_ff2 = inp("w_ff2", [DFF, D])
    fb1 = inp("fb1", [128, FC], F32)   # ff_b1 in [128, chunk] layout
    fb2 = inp("fb2", [128, DC], F32)
    lnb = inp("lnb", [128, 6 * DC], F32)  # g1,b1,g2,b2,g3,b3 packed
    outT = nc.dram_tensor("outT", [128, DC, QS], F32, kind="ExternalOutput").ap()
    dbg = nc.dram_tensor("dbg", [128, 40, QS], F32, kind="ExternalOutput").ap() \
        if tap else None
    tapped = []

    def tapit(name, ap):
        if tap and (tap == "all" or name in tap):
            tapped.append((name, ap))

    with tile.TileContext(nc) as tc:
        with tc.tile_pool(name="glob", bufs=1) as G, \
             tc.tile_pool(name="acts", bufs=2) as ACTS, \
             tc.tile_pool(name="dram", bufs=1, space="DRAM") as DRAM, \
             tc.tile_pool(name="ps2", bufs=2, space="PSUM") as PS2:

            ones_f = G.tile([128, 64], F32)
            nc.vector.memset(ones_f[:], 1.0)
            ones_b = G.tile([128, 64], BF16)
            nc.vector.tensor_copy(ones_b[:], ones_f[:])
            ones = G.tile([128, 1], F32R)
            nc.vector.tensor_copy(ones[:], ones_f[:, 0:1])
            cexpb = G.tile([128, 1], F32)
            nc.vector.memset(cexpb[:], EXP_BIAS)
            cleps = G.tile([128, 1], F32)
            nc.vector.memset(cleps[:], LN_EPS)
            lnbt = G.tile([128, 6 * DC], F32)
            nc.sync.dma_start(lnbt[:], lnb)
            fb1t = G.tile([128, FC], F32)
            nc.sync.dma_start(fb1t[:], fb1)
            fb2t = G.tile([128, DC], F32)
            nc.sync.dma_start(fb2t[:], fb2)
            vmst = G.tile([128, S // 128], F32)
            nc.sync.dma_start(vmst[:], vms)
            stats = G.tile([1, 8, QS], F32)

            def proj_from_dram(wdram, rhs, evict, n_mc=DC, n_kc=DC, wtag="w"):
                """psum[mc] = sum_kc w[kc,mc-chunk].T @ rhs[:,kc,:]; evict(mc, psum)."""
                wre = _wpairs(wdram)
                for mc in range(n_mc):
                    ps = PS2.tile([128, QS], F32, tag="pj")
                    for k2 in range(n_kc // 2):
                        wt = WPOOL.tile([128, 2, 128], BF16, tag=wtag)
                        nc.sync.dma_start(
                            wt[:], wre[:, 2 * k2:2 * k2 + 2, ds(mc * 128, 128)])
                        for j in range(2):
                            kc = 2 * k2 + j
                            nc.tensor.matmul(ps[:], wt[:, j, :], rhs[:, kc, :],
                                             start=(kc == 0), stop=(kc == n_kc - 1))
                    evict(mc, ps)

            def layernorm(xpre, gcol, bcol, out, TMP):
                """out[:,mc,:] = (xpre - mu)/sd * g + b, stats over d (partition+chunks).

                xpre must be f32r so the stats matmuls run full-rate."""
                pmu = PS2.tile([1, QS], F32, tag="pj")
                for kc in range(DC):
                    nc.tensor.matmul(pmu[:], ones[:], xpre[:, kc, :],
                                     start=(kc == 0), stop=(kc == DC - 1))
                pm2 = PS2.tile([1, QS], F32, tag="pj")
                for kc in range(DC):
                    sq = TMP.tile([128, QS], F32R, tag="sq")
                    nc.scalar.activation(sq[:], xpre[:, kc, :], AF.Square)
                    nc.tensor.matmul(pm2[:], ones[:], sq[:],
                                     start=(kc == 0), stop=(kc == DC - 1))
                mu = stats[0:1, 0, :]
                ex2 = stats[0:1, 1, :]
                var = stats[0:1, 2, :]
                sd = stats[0:1, 3, :]
                rstd = stats[0:1, 4, :]
                nc.vector.tensor_scalar_mul(mu, pmu[:], 1.0 / D)
                nc.vector.tensor_scalar_mul(ex2, pm2[:], 1.0 / D)
                nc.vector.tensor_tensor(var, mu, mu, ALU.mult)
                nc.vector.tensor_sub(var, ex2, var)
                nc.scalar.activation(sd, var, AF.Sqrt, bias=cleps[0:1, :])
                nc.vector.reciprocal(rstd, sd)
                mub = TMP.tile([128, QS], F32, tag="mub")
                nc.gpsimd.partition_broadcast(mub[:], mu)
                rsb = TMP.tile([128, QS], F32, tag="rsb")
                nc.gpsimd.partition_broadcast(rsb[:], rstd)
                for mc in range(DC):
                    t = TMP.tile([128, QS], F32, tag="t")
                    nc.vector.tensor_sub(t[:], xpre[:, mc, :], mub[:])
                    nc.vector.tensor_mul(t[:], t[:], rsb[:])
                    nc.vector.tensor_scalar(
                        out=out[:, mc, :], in0=t[:],
                        scalar1=gcol[:, mc:mc + 1], scalar2=bcol[:, mc:mc + 1],
                        op0=ALU.mult, op1=ALU.add)

            # ---- load own-panel activations ----
            xo = ACTS.tile([128, DC, QS], BF16, tag="act")
            nc.sync.dma_start(xo[:], _dchunks(xoT))

            WPOOL = None
            g1, b1 = lnbt[:, 0:DC], lnbt[:, DC:2 * DC]
            g2, b2 = lnbt[:, 2 * DC:3 * DC], lnbt[:, 3 * DC:4 * DC]
            g3, b3 = lnbt[:, 4 * DC:5 * DC], lnbt[:, 5 * DC:6 * DC]

            # ---- own-panel K/V projections + AllGather (both attentions) ----
            # Contribution layout [2*D, PANEL]: rows 0:D = K^T in d-chunk
            # layout, rows D:2D = V in kpos-major layout (row D + 2*k + dh,
            # col c holds V[k, dh*512 + c]).
            gath = {}
            with tc.tile_pool(name="kvp", bufs=1) as KVP, \
                 tc.tile_pool(name="wkv", bufs=4) as WPOOL, \
                 tc.tile_pool(name="pv", bufs=1, space="PSUM") as PV:
                eo = KVP.tile([128, DC, QS], BF16, tag="eo")
                nc.sync.dma_start(eo[:], _dchunks(eoT))
                for sfx, src, w in (("sa", xo, w_sa), ("ca", eo, w_ca)):
                    bounce = DRAM.tile([2 * D, PANEL], BF16)
                    kt_own = KVP.tile([128, DC, PANEL], BF16, tag="kt_own")

                    def evk(mc, psum, kt_own=kt_own):
                        nc.vector.tensor_copy(kt_own[:, mc, :], psum[:])
                    proj_from_dram(w["k"], src, evk, wtag="wk")
                    nc.sync.dma_start(
                        bounce[ds(0, D), :].rearrange("(c p) n -> p c n", p=128),
                        kt_own[:])

                    vo = KVP.tile([128, 2, NSC, PANEL], BF16, tag="vo")
                    wvre = _dchunks(w["v"])
                    for nh in range(2):
                        pss = [PV.tile([128, QS], F32, tag=f"pv{i}",
                                       name=f"vps{nh}{i}") for i in range(NSC)]
                        for kc in range(DC):
                            wvt = WPOOL.tile([128, PANEL], BF16, tag="wv")
                            nc.sync.dma_start(
                                wvt[:], wvre[:, kc, ds(nh * 512, 512)])
                            for sc in range(NSC):
                                nc.tensor.matmul(
                                    pss[sc][:], src[:, kc, ds(sc * 128, 128)],
                                    wvt[:], start=(kc == 0), stop=(kc == DC - 1))
                        for sc in range(NSC):
                            nc.vector.tensor_copy(vo[:, nh, sc, :], pss[sc][:])
                    nc.sync.dma_start(
                        bounce[ds(D, D), :].rearrange(
                            "(dh sc p) c -> p dh sc c", p=128, dh=2),
                        vo[:])

                    g = DRAM.tile([NPANEL * 2 * D, PANEL], BF16)
                    nc.gpsimd.collective_compute(
                        "AllGather", ALU.bypass, replica_groups=RG,
                        ins=[bounce[:]], outs=[g[:]])
                    gath[sfx] = g

            def attention(qsrc, w, gathered, res, gcol, bcol, masked):
                nonlocal WPOOL
                sfx = "sa" if masked else "ca"
                with tc.tile_pool(name="attn", bufs=1) as A:
                    QT = A.tile([128, DC, QS], BF16)
                    with tc.tile_pool(name="wq", bufs=4) as WPOOL:
                        def evq(mc, ps):
                            nc.vector.tensor_copy(QT[:, mc, :], ps[:])
                        proj_from_dram(w["q"], qsrc, evq)
                    tapit("QT" + sfx, QT)

                    # load all gathered K/V panels (+ masks) into SBUF
                    KT = A.tile([128, NPANEL, DC, PANEL], BF16)
                    v1 = A.tile([128, NPANEL, NSC, H, DK + 1], BF16)
                    for p in range(NPANEL):
                        nc.sync.dma_start(
                            KT[:, p, :, :],
                            gathered[ds(2 * D * p, D), :].rearrange(
                                "(c pp) n -> pp c n", pp=128))
                        for sc in range(NSC):
                            for dh in range(2):
                                nc.sync.dma_start(
                                    v1[:, p, sc, ds(dh * 8, 8), 0:DK],
                                    gathered[
                                        ds(2 * D * p + D + dh * 512 + sc * 128,
                                           128), :
                                    ].rearrange("pp (h8 d) -> pp h8 d", d=DK))
                        nc.vector.tensor_copy(
                            v1[:, p, :, :, DK],
                            ones_b[:].rearrange("p (a b) -> p a b", a=NSC))
                        if not masked:
                            for sc in range(NSC):
                                i = p * NSC + sc
                                nc.vector.tensor_scalar_mul(
                                    v1[:, p, sc, :, :], v1[:, p, sc, :, :],
                                    vmst[:, i:i + 1])
                    if masked:
                        mt = A.tile([128, S // 128, QS], BF16)
                        nc.sync.dma_start(mt[:], msk.rearrange("c p q -> p c q"))

                    ON = A.tile([128, DC, QS], BF16)
                    rn = A.tile([1, 2 * NHP, QS], F32)
                    with tc.tile_pool(name="pp", bufs=2) as PP, \
                         tc.tile_pool(name="rnbp", bufs=1) as RNB, \
                         tc.tile_pool(name="pso", bufs=1, space="PSUM") as PSO, \
                         tc.tile_pool(name="pss", bufs=2, space="PSUM") as PSS:
                        for hp in range(NHP):
                            po0 = PSO.tile([65, QS], F32, tag="po0")
                            po1 = PSO.tile([65, QS], F32, tag="po1")
                            for p in range(NPANEL):
                                for sc in range(NSC):
                                    ci = p * NSC + sc
                                    ps = PSS.tile([128, 2, QS], F32, tag="ps")
                                    nc.tensor.matmul(
                                        ps[:, 0, :],
                                        KT[0:64, p, hp, ds(sc * 128, 128)],
                                        QT[0:64, hp, :], start=True, stop=True)
                                    nc.tensor.matmul(
                                        ps[:, 1, :],
                                        KT[64:128, p, hp, ds(sc * 128, 128)],
                                        QT[64:128, hp, :], start=True, stop=True,
                                        tile_position=(64, 0))
                                    p01 = PP.tile([128, 2, QS], BF16, tag="p01")
                                    nc.scalar.activation(p01[:], ps[:], AF.Exp,
                                                         scale=0.125, bias=cexpb[:])
                                    if masked:
                                        nc.vector.tensor_mul(
                                            p01[:, 0, :], p01[:, 0, :], mt[:, ci, :])
                                        nc.vector.tensor_mul(
                                            p01[:, 1, :], p01[:, 1, :], mt[:, ci, :])
                                    nc.tensor.matmul(
                                        po0[:], v1[:, p, sc, 2 * hp, :],
                                        p01[:, 0, :], start=(ci == 0),
                                        stop=(ci == NPANEL * NSC - 1))
                                    nc.tensor.matmul(
                                        po1[:], v1[:, p, sc, 2 * hp + 1, :],
                                        p01[:, 1, :], start=(ci == 0),
                                        stop=(ci == NPANEL * NSC - 1))
                            # normalize: ON[:, hp] = po / po[64] (per head)
                            nrm = RNB.tile([1, 2, QS], F32, tag="nrm")
                            nc.vector.tensor_copy(nrm[0:1, 0, :], po0[64:65, :])
                            nc.vector.tensor_copy(nrm[0:1, 1, :], po1[64:65, :])
                            rnr = rn[0:1, ds(2 * hp, 2), :]
                            nc.vector.reciprocal_approx_fast(rnr, nrm[:])
                            rnb = RNB.tile([64, 2, QS], F32, tag="rnb")
                            nc.gpsimd.partition_broadcast(rnb[:, 0, :],
                                                          rn[0:1, 2 * hp, :])
                            nc.gpsimd.partition_broadcast(rnb[:, 1, :],
                                                          rn[0:1, 2 * hp + 1, :])
                            nc.vector.tensor_mul(ON[0:64, hp, :],
                                                 po0[0:64, :], rnb[:, 0, :])
                            nc.vector.tensor_mul(ON[64:128, hp, :],
                                                 po1[0:64, :], rnb[:, 1, :])
                    tapit("ON" + sfx, ON)

                    with tc.tile_pool(name="aepi", bufs=1) as E, \
                         tc.tile_pool(name="tmp", bufs=2) as TMP, \
                         tc.tile_pool(name="wo", bufs=4) as WPOOL:
                        xpre = E.tile([128, DC, QS], F32R)

                        def evo(mc, ps):
                            nc.vector.tensor_add(xpre[:, mc, :], ps[:], res[:, mc, :])
                        proj_from_dram(w["o"], ON, evo)
                        tapit("xpre" + sfx, xpre)
                        xnext = ACTS.tile([128, DC, QS], BF16, tag="act")
                        layernorm(xpre, gcol, bcol, xnext, TMP)
                        tapit("xn" + sfx, xnext)
                return xnext

            x1 = attention(xo, w_sa, gath["sa"], xo, g1, b1, masked=True)
            x2 = attention(x1, w_ca, gath["ca"], x1, g2, b2, masked=False)

            # ---- FFN ----
            with tc.tile_pool(name="ffn", bufs=1) as F, \
                 tc.tile_pool(name="tmp2", bufs=2) as TMP, \
                 tc.tile_pool(name="wf", bufs=4) as WPOOL:
                h1 = F.tile([128, FC, QS], BF16)

                def ev1(fc, ps):
                    nc.scalar.activation(h1[:, fc, :], ps[:], AF.Relu,
                                         bias=fb1t[:, fc:fc + 1])
                proj_from_dram(w_ff1, x2, ev1, n_mc=FC, n_kc=DC)

                tapit("h1a", h1[:, 0:8, :])
                xpre = F.tile([128, DC, QS], F32R)

                def ev2(mc, ps):
                    nc.vector.scalar_tensor_tensor(
                        out=xpre[:, mc, :], in0=ps[:],
                        scalar=fb2t[:, mc:mc + 1], in1=x2[:, mc, :],
                        op0=ALU.add, op1=ALU.add)
                proj_from_dram(w_ff2, h1, ev2, n_mc=DC, n_kc=FC)

                tapit("xpreff", xpre)
                out = F.tile([128, DC, QS], F32)
                layernorm(xpre, g3, b3, out, TMP)
                tapit("outf", out)
                tc.strict_bb_all_engine_barrier()
                for mc in range(DC):
                    nc.sync.dma_start(outT[:, mc, :], out[:, mc, :])
            if tap:
                base = 0
                tap_layout.clear()
                for name, t in tapped:
                    sh = t.shape
                    nparts = sh[0]
                    assert len(sh) == 3 and sh[2] == QS
                    tap_layout[name] = (base, sh[1], nparts)
                    for cci in range(sh[1]):
                        nc.sync.dma_start(
                            dbg[0:nparts, base + cci, :].bitcast(t.dtype),
                            t[:, cci, :])
                    base += sh[1]
                assert base <= 40

    nc.compile()
    return nc


_NC_CACHE = None


def _get_nc():
    global _NC_CACHE
    if _NC_CACHE is None:
        _NC_CACHE = _build()
    return _NC_CACHE


def _prep_in_maps(x, enc, tgt_mask, src_mask,
                  sa_wq, sa_wk, sa_wv, sa_wo,
                  ca_wq, ca_wk, ca_wv, ca_wo,
                  ff_w1, ff_b1, ff_w2, ff_b2,
                  ln1_g, ln1_b, ln2_g, ln2_b, ln3_g, ln3_b):
    f32 = np.float32
    bf16 = ml_dtypes.bfloat16

    def c(a):
        return np.ascontiguousarray(np.asarray(a), dtype=f32)

    def cb(a):
        return np.ascontiguousarray(np.asarray(a, dtype=f32).astype(bf16))

    xTb = [np.asarray(x, dtype=f32)[b].T.astype(bf16) for b in range(B)]
    eTb = [np.asarray(enc, dtype=f32)[b].T.astype(bf16) for b in range(B)]
    tm = np.asarray(tgt_mask)[0, 0].astype(f32).T            # [k, q]
    sm = np.asarray(src_mask)[0, 0, 0].astype(f32)           # [k]
    vms = c(sm.reshape(S // 128, 128).T)                     # [128, 16]

    def percol(v, nchunks):
        return c(np.asarray(v).reshape(nchunks, 128).T)

    lnb = c(np.concatenate(
        [percol(v, DC) for v in [ln1_g, ln1_b, ln2_g, ln2_b, ln3_g, ln3_b]],
        axis=1))
    fb1 = percol(ff_b1, FC)
    fb2 = percol(ff_b2, DC)
    shared = {
        "vms": vms, "lnb": lnb, "fb1": fb1, "fb2": fb2,
        "w_saq": cb(sa_wq), "w_sak": cb(sa_wk), "w_sav": cb(sa_wv),
        "w_sao": cb(sa_wo),
        "w_caq": cb(ca_wq), "w_cak": cb(ca_wk), "w_cav": cb(ca_wv),
        "w_cao": cb(ca_wo),
        "w_ff1": cb(ff_w1), "w_ff2": cb(ff_w2),
    }
    in_maps = []
    for core in range(NCORES):
        b, qi = core // 4, core % 4
        q0 = qi * QS
        m = dict(shared)
        m["xoT"] = np.ascontiguousarray(xTb[b][:, q0:q0 + QS])
        m["eoT"] = np.ascontiguousarray(eTb[b][:, q0:q0 + QS])
        m["msk"] = np.ascontiguousarray(
            tm[:, q0:q0 + QS].reshape(S // 128, 128, QS).astype(bf16))
        in_maps.append(m)
    return in_maps


def _gather_out(res):
    out = np.empty((B, S, D), dtype=np.float32)
    for core in range(NCORES):
        b, qi = core // 4, core % 4
        q0 = qi * QS
        arr = res.results[core]["outT"]  # [128, DC, QS]
        out[b, q0:q0 + QS, :] = arr.transpose(1, 0, 2).reshape(D, QS).T
    return out


def kernel(**inputs):
    in_maps = _prep_in_maps(**inputs)
    nc = _get_nc()
    res = run_bass_kernel_spmd(nc, in_maps, core_ids=list(range(NCORES)))
    return _gather_out(res)


def _profiled_run(inputs):
    """Test-only: run with NTFF tracing to get HW exec time."""
    in_maps = _prep_in_maps(**inputs)
    nc = _get_nc()
    return run_bass_kernel_spmd(nc, in_maps, core_ids=list(range(NCORES)),
                                trace=True)


# revision 10
# speedup vs baseline: 1.6226x; 1.2003x over previous
"""Trainium2 Bass kernel for nn_DecoderBlock (self-attn + cross-attn + FFN, post-LN).

Sharding: 8 cores = (batch b in {0,1}) x (query block qi in {0..3} of 512 rows).
Each core computes its 512 output rows end-to-end. K/V projections are
sharded: each core projects only its own 512-position panel of K and V (for
both attentions) and the panels are exchanged with AllGathers across the
4-core replica group of the batch, removing the 4x-replicated K/V projection
compute of the all-local variant. The collectives run on the TOPSP/SDMA
path and overlap with the projection compute.

All matmuls run in bfloat16 (weights are cast and repacked host-side so
every weight DMA lands as 1KB-contiguous descriptors; activations are
rounded to bf16 on chip) with fp32 PSUM accumulation; layernorm statistics
are computed on float32r copies so the stats matmuls stay full-rate.

Attention keeps activations transposed [d, s]: scores use KT chunks as the
stationary operand with two heads packed into the 128-row PE array via
tile_position; softmax is exp(s/8 - 4) with the normalizer produced by an
extra ones-column on V (M=65 matmul; the column travels through the
AllGather) and divided out after accumulation. The attention loop is
head-pair outer / panel inner so the AV accumulation stays in PSUM across
the whole sequence. Causal masking is a per-core 0/1 bf16 mask multiply on
the exp tiles; the cross-attention key mask is folded into the V rows.
"""

import numpy as np
import ml_dtypes

import concourse.bass as bass
import concourse.mybir as mybir
import concourse.tile as tile
from concourse import bacc
from concourse.bass import ds
from concourse.bass_utils import run_bass_kernel_spmd

F32 = mybir.dt.float32
F32R = mybir.dt.float32r
BF16 = mybir.dt.bfloat16
AF = mybir.ActivationFunctionType
ALU = mybir.AluOpType

B, S, D, H, DK, DFF = 2, 2048, 1024, 16, 64, 4096
NCORES = 8
QS = 512            # query rows per core
DC = D // 128       # 8 d-chunks
FC = DFF // 128     # 32 dff-chunks
PANEL = 512         # kpos panel size (= one core's contribution)
NPANEL = S // PANEL # 4
NSC = PANEL // 128  # 4 kpos chunks per panel
NHP = H // 2        # 8 head pairs
VW = H * (DK + 1)   # 1040: V panel row width incl per-head ones column
LN_EPS = 1e-5
EXP_BIAS = -4.0     # exp(s/8 - 4): overflow safety; cancels in the normalizer
RG = [[0, 1, 2, 3], [4, 5, 6, 7]]  # replica groups (one per batch)


def _dchunks(ap):
    """[D-like, N] dram AP -> [128, chunks, N] (partition = row % 128)."""
    return ap.rearrange("(c p) n -> p c n", p=128)


tap_layout = {}


def _build(tap=None):
    nc = bacc.Bacc("TRN2", target_bir_lowering=False, debug=False,
                   num_devices=NCORES)

    def inp(name, shape, dt=BF16):
        return nc.dram_tensor(name, shape, dt, kind="ExternalInput").ap()

    xoT = inp("xoT", [D, QS])          # x[b].T[:, q0:q0+QS]
    eoT = inp("eoT", [D, QS])          # enc[b].T[:, q0:q0+QS]
    msk = inp("msk", [S // 128, 128, QS])  # per-core causal mask (k-chunk, k, q)
    vms = inp("vms", [128, S // 128], F32)  # src_mask per kpos
    # packed projection weights: [n_mc, n_kc//4, 128, 512];
    # [mc, k4, p, jj*128+m] = W[(4*k4+jj)*128+p, mc*128+m]
    w_sa = {t: inp(f"w_sa{t}", [DC, DC // 4, 128, 512]) for t in "qko"}
    w_ca = {t: inp(f"w_ca{t}", [DC, DC // 4, 128, 512]) for t in "qko"}
    w_sa["v"] = inp("w_sav", [D, D])   # V proj consumes plain [K, M]
    w_ca["v"] = inp("w_cav", [D, D])
    w_ff1 = inp("w_ff1", [FC, DC // 4, 128, 512])
    w_ff2 = inp("w_ff2", [DC, FC // 4, 128, 512])
    fb1 = inp("fb1", [128, FC], F32)   # ff_b1 in [128, chunk] layout
    fb2 = inp("fb2", [128, DC], F32)
    lnb = inp("lnb", [128, 6 * DC], F32)  # g1,b1,g2,b2,g3,b3 packed
    outT = nc.dram_tensor("outT", [128, DC, QS], F32, kind="ExternalOutput").ap()
    dbg = nc.dram_tensor("dbg", [128, 40, QS], F32, kind="ExternalOutput").ap() \
        if tap else None
    tapped = []

    def tapit(name, ap):
        if tap and (tap == "all" or name in tap):
            tapped.append((name, ap))

    with tile.TileContext(nc) as tc:
        with tc.tile_pool(name="glob", bufs=1) as G, \
             tc.tile_pool(name="acts", bufs=2) as ACTS, \
             tc.tile_pool(name="dram", bufs=1, space="DRAM") as DRAM:

            ones_f = G.tile([128, 64], F32)
            nc.vector.memset(ones_f[:], 1.0)
            ones = G.tile([128, 1], F32R)
            nc.vector.tensor_copy(ones[:], ones_f[:, 0:1])
            cexpb = G.tile([128, 1], F32)
            nc.vector.memset(cexpb[:], EXP_BIAS)
            cleps = G.tile([128, 1], F32)
            nc.vector.memset(cleps[:], LN_EPS)
            lnbt = G.tile([128, 6 * DC], F32)
            nc.sync.dma_start(lnbt[:], lnb)
            fb1t = G.tile([128, FC], F32)
            nc.sync.dma_start(fb1t[:], fb1)
            fb2t = G.tile([128, DC], F32)
            nc.sync.dma_start(fb2t[:], fb2)
            vmst = G.tile([128, S // 128], F32)
            nc.sync.dma_start(vmst[:], vms)
            stats = G.tile([1, 8, QS], F32)
            # causal mask, loaded up front on the scalar queue
            mt = G.tile([128, S // 128, QS], BF16)
            nc.scalar.dma_start(mt[:], msk.rearrange("c p q -> p c q"))

            def proj_from_dram(wpk, rhs, evict, PSP, n_mc=DC, n_kc=DC,
                               wtag="w"):
                """psum[mc] = sum_kc w[kc,mc-chunk].T @ rhs[:,kc,:]; evict(mc, psum)."""
                for mc in range(n_mc):
                    ps = PSP.tile([128, QS], F32, tag="pj")
                    for k4 in range(n_kc // 4):
                        wt = WPOOL.tile([128, 4, 128], BF16, tag=wtag)
                        nc.sync.dma_start(
                            wt[:], wpk[mc, k4].rearrange("p (j m) -> p j m", j=4))
                        for j in range(4):
                            kc = 4 * k4 + j
                            nc.tensor.matmul(ps[:], wt[:, j, :], rhs[:, kc, :],
                                             start=(kc == 0), stop=(kc == n_kc - 1))
                    evict(mc, ps)

            def layernorm(xpre, gcol, bcol, out, TMP, PSP):
                """out[:,mc,:] = (xpre - mu)/sd * g + b, stats over d (partition+chunks).

                xpre must be f32r so the stats matmuls run full-rate."""
                pmu = PSP.tile([1, QS], F32, tag="pj")
                for kc in range(DC):
                    nc.tensor.matmul(pmu[:], ones[:], xpre[:, kc, :],
                                     start=(kc == 0), stop=(kc == DC - 1))
                pm2 = PSP.tile([1, QS], F32, tag="pj")
                for kc in range(DC):
                    sq = TMP.tile([128, QS], F32R, tag="sq")
                    nc.scalar.activation(sq[:], xpre[:, kc, :], AF.Square)
                    nc.tensor.matmul(pm2[:], ones[:], sq[:],
                                     start=(kc == 0), stop=(kc == DC - 1))
                mu = stats[0:1, 0, :]
                ex2 = stats[0:1, 1, :]
                var = stats[0:1, 2, :]
                sd = stats[0:1, 3, :]
                rstd = stats[0:1, 4, :]
                nc.vector.tensor_scalar_mul(mu, pmu[:], 1.0 / D)
                nc.vector.tensor_scalar_mul(ex2, pm2[:], 1.0 / D)
                nc.vector.tensor_tensor(var, mu, mu, ALU.mult)
                nc.vector.tensor_sub(var, ex2, var)
                nc.scalar.activation(sd, var, AF.Sqrt, bias=cleps[0:1, :])
                nc.vector.reciprocal(rstd, sd)
                mub = TMP.tile([128, QS], F32, tag="mub")
                nc.gpsimd.partition_broadcast(mub[:], mu)
                rsb = TMP.tile([128, QS], F32, tag="rsb")
                nc.gpsimd.partition_broadcast(rsb[:], rstd)
                for mc in range(DC):
                    t = TMP.tile([128, QS], F32, tag="t")
                    nc.vector.tensor_sub(t[:], xpre[:, mc, :], mub[:])
                    nc.vector.tensor_mul(t[:], t[:], rsb[:])
                    nc.vector.tensor_scalar(
                        out=out[:, mc, :], in0=t[:],
                        scalar1=gcol[:, mc:mc + 1], scalar2=bcol[:, mc:mc + 1],
                        op0=ALU.mult, op1=ALU.add)

            # ---- load own-panel activations ----
            xo = ACTS.tile([128, DC, QS], BF16, tag="act")
            nc.sync.dma_start(xo[:], _dchunks(xoT))

            WPOOL = None
            g1, b1 = lnbt[:, 0:DC], lnbt[:, DC:2 * DC]
            g2, b2 = lnbt[:, 2 * DC:3 * DC], lnbt[:, 3 * DC:4 * DC]
            g3, b3 = lnbt[:, 4 * DC:5 * DC], lnbt[:, 5 * DC:6 * DC]

            # ---- own-panel K/V projections + AllGathers (both attentions) ----
            # K bounce [D, PANEL]: row c*128+p holds K^T[d=c*128+p, k].
            # V bounce [D, VW...]: row dh*512+sc*128+pp, col h8*65+e holds
            # V[k=sc*128+pp, (dh*8+h8)*64+e] for e<64, and 1.0 at e=64.
            gath = {}
            with tc.tile_pool(name="kvp", bufs=1) as KVP, \
                 tc.tile_pool(name="wkv", bufs=4) as WPOOL, \
                 tc.tile_pool(name="psk", bufs=2, space="PSUM") as PSK, \
                 tc.tile_pool(name="pv", bufs=1, space="PSUM") as PV:
                eo = KVP.tile([128, DC, QS], BF16, tag="eo")
                nc.sync.dma_start(eo[:], _dchunks(eoT))
                for sfx, src, w in (("sa", xo, w_sa), ("ca", eo, w_ca)):
                    kb = DRAM.tile([D, PANEL], BF16)
                    vb = DRAM.tile([D, VW // 2], BF16)
                    kt_own = KVP.tile([128, DC, PANEL], BF16, tag="kt_own")

                    def evk(mc, psum, kt_own=kt_own):
                        nc.vector.tensor_copy(kt_own[:, mc, :], psum[:])
                    proj_from_dram(w["k"], src, evk, PSK, wtag="wk")
                    nc.sync.dma_start(
                        kb.rearrange("(c p) n -> p c n", p=128), kt_own[:])
                    kg = DRAM.tile([NPANEL * D, PANEL], BF16)
                    nc.gpsimd.collective_compute(
                        "AllGather", ALU.bypass, replica_groups=RG,
                        ins=[kb[:]], outs=[kg[:]])

                    vo = KVP.tile([128, 2, NSC, VW // 2], BF16, tag="vo")
                    nc.vector.memset(
                        vo.rearrange("p a s (h e) -> p a s h e", e=DK + 1)
                        [:, :, :, :, DK], 1.0)
                    wvre = _dchunks(w["v"])
                    for nh in range(2):
                        pss = [PV.tile([128, QS], F32, tag=f"pv{i}",
                                       name=f"vps{nh}{i}") for i in range(NSC)]
                        for kc in range(DC):
                            wvt = WPOOL.tile([128, PANEL], BF16, tag="wv")
                            nc.sync.dma_start(
                                wvt[:], wvre[:, kc, ds(nh * 512, 512)])
                            for sc in range(NSC):
                                nc.tensor.matmul(
                                    pss[sc][:], src[:, kc, ds(sc * 128, 128)],
                                    wvt[:], start=(kc == 0), stop=(kc == DC - 1))
                        for sc in range(NSC):
                            nc.vector.tensor_copy(
                                vo[:, nh, sc, :].rearrange(
                                    "p (h e) -> p h e", e=DK + 1)[:, :, 0:DK],
                                pss[sc][:].rearrange("p (h d) -> p h d", d=DK))
                    nc.sync.dma_start(
                        vb.rearrange("(dh sc p) c -> p dh sc c", p=128, dh=2),
                        vo[:])
                    vg = DRAM.tile([NPANEL * D, VW // 2], BF16)
                    nc.gpsimd.collective_compute(
                        "AllGather", ALU.bypass, replica_groups=RG,
                        ins=[vb[:]], outs=[vg[:]])
                    gath[sfx] = (kg, vg)

            def attention(qsrc, w, gathered, res, gcol, bcol, masked):
                nonlocal WPOOL
                sfx = "sa" if masked else "ca"
                kg, vg = gathered
                with tc.tile_pool(name="attn", bufs=1) as A:
                    QT = A.tile([128, DC, QS], BF16)
                    with tc.tile_pool(name="wq", bufs=4) as WPOOL, \
                         tc.tile_pool(name="psq", bufs=2, space="PSUM") as PSQ:
                        def evq(mc, ps):
                            nc.vector.tensor_copy(QT[:, mc, :], ps[:])
                        proj_from_dram(w["q"], qsrc, evq, PSQ)
                    tapit("QT" + sfx, QT)

                    # load all gathered K/V panels into SBUF (scalar queue)
                    KT = A.tile([128, NPANEL, DC, PANEL], BF16)
                    v1 = A.tile([128, NPANEL, NSC, H, DK + 1], BF16)
                    for p in range(NPANEL):
                        nc.scalar.dma_start(
                            KT[:, p, :, :],
                            kg[ds(D * p, D), :].rearrange(
                                "(c pp) n -> pp c n", pp=128))
                        for dh in range(2):
                            nc.scalar.dma_start(
                                v1[:, p, :, ds(dh * 8, 8), :].rearrange(
                                    "pp sc h e -> pp sc (h e)"),
                                vg[ds(D * p + dh * 512, 512), :].rearrange(
                                    "(sc pp) c -> pp sc c", pp=128))
                        if not masked:
                            for sc in range(NSC):
                                i = p * NSC + sc
                                nc.vector.tensor_scalar_mul(
                                    v1[:, p, sc, :, :], v1[:, p, sc, :, :],
                                    vmst[:, i:i + 1])

                    ON = A.tile([128, DC, QS], BF16)
                    rn = A.tile([1, 2 * NHP, QS], F32)
                    with tc.tile_pool(name="pp", bufs=2) as PP, \
                         tc.tile_pool(name="rnbp", bufs=1) as RNB, \
                         tc.tile_pool(name="pso", bufs=2, space="PSUM") as PSO, \
                         tc.tile_pool(name="pss", bufs=2, space="PSUM") as PSS:
                        for hp in range(NHP):
                            po0 = PSO.tile([65, QS], F32, tag="po0")
                            po1 = PSO.tile([65, QS], F32, tag="po1")
                            for p in range(NPANEL):
                                for sc in range(NSC):
                                    ci = p * NSC + sc
                                    ps = PSS.tile([128, 2, QS], F32, tag="ps")
                                    nc.tensor.matmul(
                                        ps[:, 0, :],
                                        KT[0:64, p, hp, ds(sc * 128, 128)],
                                        QT[0:64, hp, :], start=True, stop=True)
                                    nc.tensor.matmul(
                                        ps[:, 1, :],
                                        KT[64:128, p, hp, ds(sc * 128, 128)],
                                        QT[64:128, hp, :], start=True, stop=True,
                                        tile_position=(64, 0))
                                    p01 = PP.tile([128, 2, QS], BF16, tag="p01")
                                    nc.scalar.activation(p01[:], ps[:], AF.Exp,
                                                         scale=0.125, bias=cexpb[:])
                                    if masked:
                                        nc.vector.tensor_mul(
                                            p01[:, 0, :], p01[:, 0, :], mt[:, ci, :])
                                        nc.vector.tensor_mul(
                                            p01[:, 1, :], p01[:, 1, :], mt[:, ci, :])
                                    nc.tensor.matmul(
                                        po0[:], v1[:, p, sc, 2 * hp, :],
                                        p01[:, 0, :], start=(ci == 0),
                                        stop=(ci == NPANEL * NSC - 1))
                                    nc.tensor.matmul(
                                        po1[:], v1[:, p, sc, 2 * hp + 1, :],
                                        p01[:, 1, :], start=(ci == 0),
                                        stop=(ci == NPANEL * NSC - 1))
                            # normalize: ON[:, hp] = po / po[64] (per head)
                            nrm = RNB.tile([1, 2, QS], F32, tag="nrm")
                            nc.vector.tensor_copy(nrm[0:1, 0, :], po0[64:65, :])
                            nc.vector.tensor_copy(nrm[0:1, 1, :], po1[64:65, :])
                            rnr = rn[0:1, ds(2 * hp, 2), :]
                            nc.vector.reciprocal_approx_fast(rnr, nrm[:])
                            rnb = RNB.tile([64, 2, QS], F32, tag="rnb")
                            nc.gpsimd.partition_broadcast(rnb[:, 0, :],
                                                          rn[0:1, 2 * hp, :])
                            nc.gpsimd.partition_broadcast(rnb[:, 1, :],
                                                          rn[0:1, 2 * hp + 1, :])
                            nc.vector.tensor_mul(ON[0:64, hp, :],
                                                 po0[0:64, :], rnb[:, 0, :])
                            nc.vector.tensor_mul(ON[64:128, hp, :],
                                                 po1[0:64, :], rnb[:, 1, :])
                    tapit("ON" + sfx, ON)

                    with tc.tile_pool(name="aepi", bufs=1) as E, \
                         tc.tile_pool(name="tmp", bufs=2) as TMP, \
                         tc.tile_pool(name="pse", bufs=2, space="PSUM") as PSE, \
                         tc.tile_pool(name="wo", bufs=4) as WPOOL:
                        xpre = E.tile([128, DC, QS], F32R)

                        def evo(mc, ps):
                            nc.vector.tensor_add(xpre[:, mc, :], ps[:], res[:, mc, :])
                        proj_from_dram(w["o"], ON, evo, PSE)
                        tapit("xpre" + sfx, xpre)
                        xnext = ACTS.tile([128, DC, QS], BF16, tag="act")
                        layernorm(xpre, gcol, bcol, xnext, TMP, PSE)
                        tapit("xn" + sfx, xnext)
                return xnext

            x1 = attention(xo, w_sa, gath["sa"], xo, g1, b1, masked=True)
            x2 = attention(x1, w_ca, gath["ca"], x1, g2, b2, masked=False)

            # ---- FFN ----
            with tc.tile_pool(name="ffn", bufs=1) as F, \
                 tc.tile_pool(name="tmp2", bufs=2) as TMP, \
                 tc.tile_pool(name="psf", bufs=2, space="PSUM") as PSF, \
                 tc.tile_pool(name="wf", bufs=4) as WPOOL:
                h1 = F.tile([128, FC, QS], BF16)

                def ev1(fc, ps):
                    nc.scalar.activation(h1[:, fc, :], ps[:], AF.Relu,
                                         bias=fb1t[:, fc:fc + 1])
                proj_from_dram(w_ff1, x2, ev1, PSF, n_mc=FC, n_kc=DC)

                tapit("h1a", h1[:, 0:8, :])
                xpre = F.tile([128, DC, QS], F32R)

                def ev2(mc, ps):
                    nc.vector.scalar_tensor_tensor(
                        out=xpre[:, mc, :], in0=ps[:],
                        scalar=fb2t[:, mc:mc + 1], in1=x2[:, mc, :],
                        op0=ALU.add, op1=ALU.add)
                proj_from_dram(w_ff2, h1, ev2, PSF, n_mc=DC, n_kc=FC)

                tapit("xpreff", xpre)
                out = F.tile([128, DC, QS], F32)
                layernorm(xpre, g3, b3, out, TMP, PSF)
                tapit("outf", out)
                tc.strict_bb_all_engine_barrier()
                for mc in range(DC):
                    nc.sync.dma_start(outT[:, mc, :], out[:, mc, :])
            if tap:
                base = 0
                tap_layout.clear()
                for name, t in tapped:
                    sh = t.shape
                    nparts = sh[0]
                    assert len(sh) == 3 and sh[2] == QS
                    tap_layout[name] = (base, sh[1], nparts)
                    for cci in range(sh[1]):
                        nc.sync.dma_start(
                            dbg[0:nparts, base + cci, :].bitcast(t.dtype),
                            t[:, cci, :])
                    base += sh[1]
                assert base <= 40

    nc.compile()
    return nc


_NC_CACHE = None


def _get_nc():
    global _NC_CACHE
    if _NC_CACHE is None:
        _NC_CACHE = _build()
    return _NC_CACHE


def _pack_w(w):
    """[K, M] fp32 -> packed bf16 [M//128, K//512, 128, 512] (see _build)."""
    K, M = w.shape
    nk4, nmc = K // 512, M // 128
    p = w.reshape(nk4, 4, 128, nmc, 128).transpose(3, 0, 2, 1, 4)
    return np.ascontiguousarray(p.reshape(nmc, nk4, 128, 512)
                                .astype(ml_dtypes.bfloat16))


def _prep_in_maps(x, enc, tgt_mask, src_mask,
                  sa_wq, sa_wk, sa_wv, sa_wo,
                  ca_wq, ca_wk, ca_wv, ca_wo,
                  ff_w1, ff_b1, ff_w2, ff_b2,
                  ln1_g, ln1_b, ln2_g, ln2_b, ln3_g, ln3_b):
    f32 = np.float32
    bf16 = ml_dtypes.bfloat16

    def c(a):
        return np.ascontiguousarray(np.asarray(a), dtype=f32)

    def cb(a):
        return np.ascontiguousarray(np.asarray(a, dtype=f32).astype(bf16))

    xTb = [np.asarray(x, dtype=f32)[b].T.astype(bf16) for b in range(B)]
    eTb = [np.asarray(enc, dtype=f32)[b].T.astype(bf16) for b in range(B)]
    tm = np.asarray(tgt_mask)[0, 0].astype(f32).T            # [k, q]
    sm = np.asarray(src_mask)[0, 0, 0].astype(f32)           # [k]
    vms = c(sm.reshape(S // 128, 128).T)                     # [128, 16]

    def percol(v, nchunks):
        return c(np.asarray(v).reshape(nchunks, 128).T)

    lnb = c(np.concatenate(
        [percol(v, DC) for v in [ln1_g, ln1_b, ln2_g, ln2_b, ln3_g, ln3_b]],
        axis=1))
    fb1 = percol(ff_b1, FC)
    fb2 = percol(ff_b2, DC)
    shared = {
        "vms": vms, "lnb": lnb, "fb1": fb1, "fb2": fb2,
        "w_saq": _pack_w(c(sa_wq)), "w_sak": _pack_w(c(sa_wk)),
        "w_sao": _pack_w(c(sa_wo)), "w_sav": cb(sa_wv),
        "w_caq": _pack_w(c(ca_wq)), "w_cak": _pack_w(c(ca_wk)),
        "w_cao": _pack_w(c(ca_wo)), "w_cav": cb(ca_wv),
        "w_ff1": _pack_w(c(ff_w1)), "w_ff2": _pack_w(c(ff_w2)),
    }
    in_maps = []
    for core in range(NCORES):
        b, qi = core // 4, core % 4
        q0 = qi * QS
        m = dict(shared)
        m["xoT"] = np.ascontiguousarray(xTb[b][:, q0:q0 + QS])
        m["eoT"] = np.ascontiguousarray(eTb[b][:, q0:q0 + QS])
        m["msk"] = np.ascontiguousarray(
            tm[:, q0:q0 + QS].reshape(S // 128, 128, QS).astype(bf16))
        in_maps.append(m)
    return in_maps


def _gather_out(res):
    out = np.empty((B, S, D), dtype=np.float32)
    for core in range(NCORES):
        b, qi = core // 4, core % 4
        q0 = qi * QS
        arr = res.results[core]["outT"]  # [128, DC, QS]
        out[b, q0:q0 + QS, :] = arr.transpose(1, 0, 2).reshape(D, QS).T
    return out


def kernel(**inputs):
    in_maps = _prep_in_maps(**inputs)
    nc = _get_nc()
    res = run_bass_kernel_spmd(nc, in_maps, core_ids=list(range(NCORES)))
    return _gather_out(res)


def _profiled_run(inputs):
    """Test-only: run with NTFF tracing to get HW exec time."""
    in_maps = _prep_in_maps(**inputs)
    nc = _get_nc()
    return run_bass_kernel_spmd(nc, in_maps, core_ids=list(range(NCORES)),
                                trace=True)


# revision 14
# speedup vs baseline: 1.7200x; 1.0600x over previous
"""Trainium2 Bass kernel for nn_DecoderBlock (self-attn + cross-attn + FFN, post-LN).

Sharding: 8 cores = (batch b in {0,1}) x (query block qi in {0..3} of 512 rows).
Each core computes its 512 output rows end-to-end. K/V projections are
sharded: each core projects only its own 512-position panel of K and V (for
both attentions) and the panels are exchanged with AllGathers across the
4-core replica group of the batch, removing the 4x-replicated K/V projection
compute of the all-local variant. The collectives run on the TOPSP/SDMA
path; the weights needed while their traffic saturates the DMA engines
(SA-Q, CA-K, CA-V) are prefetched into SBUF before the first collective
fires, and the CA K/V panels are loaded during the SA attention phase.

All matmuls run in bfloat16 (weights are cast and repacked host-side so
every weight DMA lands as 1KB-contiguous descriptors; activations are
rounded to bf16 on chip) with fp32 PSUM accumulation; layernorm statistics
are computed on float32r copies so the stats matmuls stay full-rate.

Attention keeps activations transposed [d, s]: scores use KT chunks as the
stationary operand with two heads packed into the 128-row PE array via
tile_position; softmax is exp(s/8 - 4) with the normalizer produced by an
extra ones-column on V (M=65 matmul; the column travels through the
AllGather) and divided out after accumulation. The attention loop is
head-pair outer / panel inner so the AV accumulation stays in PSUM across
the whole sequence. Causal masking is a per-core 0/1 bf16 mask multiply on
the exp tiles; the cross-attention key mask is folded into the V rows.
"""

import numpy as np
import ml_dtypes

import concourse.bass as bass
import concourse.mybir as mybir
import concourse.tile as tile
from concourse import bacc
from concourse.bass import ds
from concourse.bass_utils import run_bass_kernel_spmd

F32 = mybir.dt.float32
F32R = mybir.dt.float32r
BF16 = mybir.dt.bfloat16
FP8 = mybir.dt.float8e3
AF = mybir.ActivationFunctionType
ALU = mybir.AluOpType

B, S, D, H, DK, DFF = 2, 2048, 1024, 16, 64, 4096
NCORES = 8
QS = 512            # query rows per core
DC = D // 128       # 8 d-chunks
FC = DFF // 128     # 32 dff-chunks
PANEL = 512         # kpos panel size (= one core's contribution)
NPANEL = S // PANEL # 4
NSC = PANEL // 128  # 4 kpos chunks per panel
NHP = H // 2        # 8 head pairs
VW = H * (DK + 1)   # 1040: V panel row width incl per-head ones column
LN_EPS = 1e-5
EXP_BIAS = -4.0     # exp(s/8 - 4): overflow safety; cancels in the normalizer
RG = [[0, 1, 2, 3], [4, 5, 6, 7]]  # replica groups (one per batch)


def _dchunks(ap):
    """[D-like, N] dram AP -> [128, chunks, N] (partition = row % 128)."""
    return ap.rearrange("(c p) n -> p c n", p=128)


tap_layout = {}


def _build(tap=None):
    nc = bacc.Bacc("TRN2", target_bir_lowering=False, debug=False,
                   num_devices=NCORES)

    def inp(name, shape, dt=BF16):
        return nc.dram_tensor(name, shape, dt, kind="ExternalInput").ap()

    xoT = inp("xoT", [D, QS])          # x[b].T[:, q0:q0+QS]
    eoT = inp("eoT", [D, QS])          # enc[b].T[:, q0:q0+QS]
    msk = inp("msk", [S // 128, 128, QS])  # per-core causal mask (k-chunk, k, q)
    vms = inp("vms", [128, S // 128], F32)  # src_mask per kpos
    # packed projection weights: [n_mc, n_kc//4, 128, 512];
    # [mc, k4, p, jj*128+m] = W[(4*k4+jj)*128+p, mc*128+m]
    w_sa = {t: inp(f"w_sa{t}", [DC, DC // 4, 128, 512]) for t in "qko"}
    w_ca = {t: inp(f"w_ca{t}", [DC, DC // 4, 128, 512]) for t in "qko"}
    w_sa["v"] = inp("w_sav", [D, D])   # V proj consumes plain [K, M]
    w_ca["v"] = inp("w_cav", [D, D])
    w_ff1 = inp("w_ff1", [FC, DC // 4, 128, 512])
    w_ff2 = inp("w_ff2", [DC, FC // 4, 128, 512])
    fb1 = inp("fb1", [128, FC], F32)   # ff_b1 in [128, chunk] layout
    fb2 = inp("fb2", [128, DC], F32)
    lnb = inp("lnb", [128, 6 * DC], F32)  # g1,b1,g2,b2,g3,b3 packed
    outT = nc.dram_tensor("outT", [128, DC, QS], F32, kind="ExternalOutput").ap()
    dbg = nc.dram_tensor("dbg", [128, 40, QS], F32, kind="ExternalOutput").ap() \
        if tap else None
    tapped = []

    def tapit(name, ap):
        if tap and (tap == "all" or name in tap):
            tapped.append((name, ap))

    with tile.TileContext(nc) as tc:
        with tc.tile_pool(name="glob", bufs=1) as G, \
             tc.tile_pool(name="acts", bufs=2) as ACTS, \
             tc.tile_pool(name="dram", bufs=1, space="DRAM") as DRAM:

            ones_f = G.tile([128, 64], F32)
            nc.vector.memset(ones_f[:], 1.0)
            ones = G.tile([128, 1], F32R)
            nc.vector.tensor_copy(ones[:], ones_f[:, 0:1])
            cexpb = G.tile([128, 1], F32)
            nc.vector.memset(cexpb[:], EXP_BIAS)
            cleps = G.tile([128, 1], F32)
            nc.vector.memset(cleps[:], LN_EPS)
            lnbt = G.tile([128, 6 * DC], F32)
            nc.sync.dma_start(lnbt[:], lnb)
            fb1t = G.tile([128, FC], F32)
            nc.sync.dma_start(fb1t[:], fb1)
            fb2t = G.tile([128, DC], F32)
            nc.sync.dma_start(fb2t[:], fb2)
            vmst = G.tile([128, S // 128], F32)
            nc.sync.dma_start(vmst[:], vms)

            WPOOL = None

            def proj_from_dram(wpk, rhs, evict, PSP, n_mc=DC, n_kc=DC,
                               wtag="w"):
                """psum[mc] = sum_kc w[kc,mc-chunk].T @ rhs[:,kc,:]; evict(mc, psum)."""
                for mc in range(n_mc):
                    ps = PSP.tile([128, QS], F32, tag="pj")
                    for k4 in range(n_kc // 4):
                        wt = WPOOL.tile([128, 4, 128], BF16, tag=wtag)
                        nc.sync.dma_start(
                            wt[:], wpk[mc, k4].rearrange("p (j m) -> p j m", j=4))
                        for j in range(4):
                            kc = 4 * k4 + j
                            nc.tensor.matmul(ps[:], wt[:, j, :], rhs[:, kc, :],
                                             start=(kc == 0), stop=(kc == n_kc - 1))
                    evict(mc, ps)

            def proj_from_sbuf(wt, rhs, evict, PSP, n_mc=DC, n_kc=DC):
                """Same as proj_from_dram but with SBUF-resident packed weights
                wt [128, n_mc, n_kc//4, 512]."""
                for mc in range(n_mc):
                    ps = PSP.tile([128, QS], F32, tag="pj")
                    for kc in range(n_kc):
                        k4, j = kc // 4, kc % 4
                        nc.tensor.matmul(ps[:], wt[:, mc, k4, ds(j * 128, 128)],
                                         rhs[:, kc, :],
                                         start=(kc == 0), stop=(kc == n_kc - 1))
                    evict(mc, ps)

            def layernorm(xpre, gcol, bcol, out, TMP, PSP):
                """out[:,mc,:] = (xpre - mu)/sd * g + b, stats over d (partition+chunks).

                xpre must be f32r so the stats matmuls run full-rate."""
                pmu = PSP.tile([1, QS], F32, tag="pj")
                for kc in range(DC):
                    nc.tensor.matmul(pmu[:], ones[:], xpre[:, kc, :],
                                     start=(kc == 0), stop=(kc == DC - 1))
                pm2 = PSP.tile([1, QS], F32, tag="pj")
                for kc in range(DC):
                    sq = TMP.tile([128, QS], F32R, tag="sq")
                    nc.scalar.activation(sq[:], xpre[:, kc, :], AF.Square)
                    nc.tensor.matmul(pm2[:], ones[:], sq[:],
                                     start=(kc == 0), stop=(kc == DC - 1))
                st = TMP.tile([1, 5, QS], F32, tag="st")
                mu = st[0:1, 0, :]
                ex2 = st[0:1, 1, :]
                var = st[0:1, 2, :]
                sd = st[0:1, 3, :]
                rstd = st[0:1, 4, :]
                nc.vector.tensor_scalar_mul(mu, pmu[:], 1.0 / D)
                nc.vector.tensor_scalar_mul(ex2, pm2[:], 1.0 / D)
                nc.vector.tensor_tensor(var, mu, mu, ALU.mult)
                nc.vector.tensor_sub(var, ex2, var)
                nc.scalar.activation(sd, var, AF.Sqrt, bias=cleps[0:1, :])
                nc.vector.reciprocal(rstd, sd)
                mub = TMP.tile([128, QS], F32, tag="mub")
                nc.gpsimd.partition_broadcast(mub[:], mu)
                rsb = TMP.tile([128, QS], F32, tag="rsb")
                nc.gpsimd.partition_broadcast(rsb[:], rstd)
                for mc in range(DC):
                    t = TMP.tile([128, QS], F32, tag="t")
                    nc.vector.tensor_sub(t[:], xpre[:, mc, :], mub[:])
                    nc.vector.tensor_mul(t[:], t[:], rsb[:])
                    nc.vector.tensor_scalar(
                        out=out[:, mc, :], in0=t[:],
                        scalar1=gcol[:, mc:mc + 1], scalar2=bcol[:, mc:mc + 1],
                        op0=ALU.mult, op1=ALU.add)

            def kv_panel_proj(src, w, kb, vb, PSK, PV):
                """Project own K/V panel from src, write bounce tensors."""
                kt_own = KVP.tile([128, DC, PANEL], FP8, tag="kt_own")

                def evk(mc, psum):
                    nc.vector.tensor_copy(kt_own[:, mc, :], psum[:])
                if isinstance(w["k"], tuple):
                    proj_from_sbuf(w["k"][0], src, evk, PSK)
                else:
                    proj_from_dram(w["k"], src, evk, PSK, wtag="wk")
                nc.sync.dma_start(
                    kb.rearrange("(c p) n -> p c n", p=128), kt_own[:])

                vo = KVP.tile([128, 2, NSC, VW // 2], FP8, tag="vo")
                nc.vector.memset(
                    vo.rearrange("p a s (h e) -> p a s h e", e=DK + 1)
                    [:, :, :, :, DK], 1.0)
                for nh in range(2):
                    pss = [PV.tile([128, QS], F32, tag=f"pv{i}", name=f"vps{nh}{i}")
                           for i in range(NSC)]
                    for kc in range(DC):
                        if isinstance(w["v"], tuple):
                            wvt = w["v"][0][:, kc, ds(nh * 512, 512)]
                        else:
                            wvtile = WPOOL.tile([128, PANEL], BF16, tag="wv")
                            nc.sync.dma_start(
                                wvtile[:],
                                _dchunks(w["v"])[:, kc, ds(nh * 512, 512)])
                            wvt = wvtile[:]
                        for sc in range(NSC):
                            nc.tensor.matmul(
                                pss[sc][:], src[:, kc, ds(sc * 128, 128)],
                                wvt, start=(kc == 0), stop=(kc == DC - 1))
                    for sc in range(NSC):
                        nc.vector.tensor_copy(
                            vo[:, nh, sc, :].rearrange(
                                "p (h e) -> p h e", e=DK + 1)[:, :, 0:DK],
                            pss[sc][:].rearrange("p (h d) -> p h d", d=DK))
                nc.sync.dma_start(
                    vb.rearrange("(dh sc p) c -> p dh sc c", p=128, dh=2),
                    vo[:])

            def ag(bounce, shape):
                g = DRAM.tile(shape, FP8)
                nc.gpsimd.collective_compute(
                    "AllGather", ALU.bypass, replica_groups=RG,
                    ins=[bounce[:]], outs=[g[:]])
                return g

            def load_panels(KT, v1, kg, vg, eng, masked):
                for p in range(NPANEL):
                    eng.dma_start(
                        KT[:, p, :, :],
                        kg[ds(D * p, D), :].rearrange("(c pp) n -> pp c n",
                                                      pp=128))
                    for dh in range(2):
                        eng.dma_start(
                            v1[:, p, :, ds(dh * 8, 8), :].rearrange(
                                "pp sc h e -> pp sc (h e)"),
                            vg[ds(D * p + dh * 512, 512), :].rearrange(
                                "(sc pp) c -> pp sc c", pp=128))
                    if not masked:
                        for sc in range(NSC):
                            i = p * NSC + sc
                            nc.vector.tensor_scalar_mul(
                                v1[:, p, sc, :, :], v1[:, p, sc, :, :],
                                vmst[:, i:i + 1])

            def attention_inner(QT, KT, v1, mt, ON, masked):
                with tc.tile_pool(name="pp", bufs=2) as PP, \
                     tc.tile_pool(name="rnbp", bufs=1) as RNB, \
                     tc.tile_pool(name="pso", bufs=2, space="PSUM") as PSO, \
                     tc.tile_pool(name="pss", bufs=2, space="PSUM") as PSS:
                    for hp in range(NHP):
                        po0 = PSO.tile([65, QS], F32, tag="po0")
                        po1 = PSO.tile([65, QS], F32, tag="po1")
                        for p in range(NPANEL):
                            for sc in range(NSC):
                                ci = p * NSC + sc
                                ps = PSS.tile([128, 2, QS], F32, tag="ps")
                                nc.tensor.matmul(
                                    ps[:, 0, :],
                                    KT[0:64, p, hp, ds(sc * 128, 128)],
                                    QT[0:64, hp, :], start=True, stop=True)
                                nc.tensor.matmul(
                                    ps[:, 1, :],
                                    KT[64:128, p, hp, ds(sc * 128, 128)],
                                    QT[64:128, hp, :], start=True, stop=True,
                                    tile_position=(64, 0))
                                p01 = PP.tile([128, 2, QS], BF16, tag="p01")
                                nc.scalar.activation(p01[:], ps[:], AF.Exp,
                                                     scale=0.125, bias=cexpb[:])
                                if masked:
                                    nc.vector.tensor_mul(
                                        p01[:, 0, :], p01[:, 0, :], mt[:, ci, :])
                                    nc.vector.tensor_mul(
                                        p01[:, 1, :], p01[:, 1, :], mt[:, ci, :])
                                nc.tensor.matmul(
                                    po0[:], v1[:, p, sc, 2 * hp, :],
                                    p01[:, 0, :], start=(ci == 0),
                                    stop=(ci == NPANEL * NSC - 1))
                                nc.tensor.matmul(
                                    po1[:], v1[:, p, sc, 2 * hp + 1, :],
                                    p01[:, 1, :], start=(ci == 0),
                                    stop=(ci == NPANEL * NSC - 1))
                        # normalize: ON[:, hp] = po / po[64] (per head)
                        nrm = RNB.tile([1, 2, QS], F32, tag="nrm")
                        nc.vector.tensor_copy(nrm[0:1, 0, :], po0[64:65, :])
                        nc.vector.tensor_copy(nrm[0:1, 1, :], po1[64:65, :])
                        rr = RNB.tile([1, 2, QS], F32, tag="rr")
                        nc.vector.reciprocal_approx_fast(rr[:], nrm[:])
                        rnb = RNB.tile([64, 2, QS], F32, tag="rnb")
                        nc.gpsimd.partition_broadcast(rnb[:, 0, :], rr[0:1, 0, :])
                        nc.gpsimd.partition_broadcast(rnb[:, 1, :], rr[0:1, 1, :])
                        nc.vector.tensor_mul(ON[0:64, hp, :],
                                             po0[0:64, :], rnb[:, 0, :])
                        nc.vector.tensor_mul(ON[64:128, hp, :],
                                             po1[0:64, :], rnb[:, 1, :])

            def attn_epilogue(w_o, ON, res, gcol, bcol, sfx):
                nonlocal WPOOL
                with tc.tile_pool(name="aepi", bufs=1) as E, \
                     tc.tile_pool(name="tmp", bufs=2) as TMP, \
                     tc.tile_pool(name="pse", bufs=2, space="PSUM") as PSE, \
                     tc.tile_pool(name="wo", bufs=6) as WPOOL:
                    xpre = E.tile([128, DC, QS], F32R)

                    def evo(mc, ps):
                        nc.vector.tensor_add(xpre[:, mc, :], ps[:], res[:, mc, :])
                    proj_from_dram(w_o, ON, evo, PSE)
                    tapit("xpre" + sfx, xpre)
                    xnext = ACTS.tile([128, DC, QS], BF16, tag="act")
                    layernorm(xpre, gcol, bcol, xnext, TMP, PSE)
                    tapit("xn" + sfx, xnext)
                return xnext

            # ---- load own-panel activations ----
            xo = ACTS.tile([128, DC, QS], BF16, tag="act")
            nc.sync.dma_start(xo[:], _dchunks(xoT))

            g1, b1 = lnbt[:, 0:DC], lnbt[:, DC:2 * DC]
            g2, b2 = lnbt[:, 2 * DC:3 * DC], lnbt[:, 3 * DC:4 * DC]
            g3, b3 = lnbt[:, 4 * DC:5 * DC], lnbt[:, 5 * DC:6 * DC]

            kb_sa = DRAM.tile([D, PANEL], FP8)
            vb_sa = DRAM.tile([D, VW // 2], FP8)
            kb_ca = DRAM.tile([D, PANEL], FP8)
            vb_ca = DRAM.tile([D, VW // 2], FP8)

            with tc.tile_pool(name="attn_ca", bufs=1) as A_CA:
                with tc.tile_pool(name="attn_sa", bufs=1) as A_SA:
                    mt = A_SA.tile([128, S // 128, QS], BF16)
                    nc.scalar.dma_start(mt[:], msk.rearrange("c p q -> p c q"))
                    QT_sa = A_SA.tile([128, DC, QS], BF16)

                    # ---- K/V panel projections + AllGathers ----
                    with tc.tile_pool(name="pref", bufs=1) as PRE, \
                         tc.tile_pool(name="kvp", bufs=1) as KVP, \
                         tc.tile_pool(name="wkv", bufs=6) as WPOOL, \
                         tc.tile_pool(name="psk", bufs=2, space="PSUM") as PSK, \
                         tc.tile_pool(name="pv", bufs=1, space="PSUM") as PV:
                        # prefetch the weights whose streaming would collide
                        # with collective traffic (vector queue)
                        wqs = PRE.tile([128, DC, DC // 4, 512], BF16)
                        nc.gpsimd.dma_start(
                            wqs[:], w_sa["q"].rearrange("a b p c -> p a b c"))
                        wck = PRE.tile([128, DC, DC // 4, 512], BF16)
                        nc.gpsimd.dma_start(
                            wck[:], w_ca["k"].rearrange("a b p c -> p a b c"))

                        eo = KVP.tile([128, DC, QS], BF16, tag="eo")
                        nc.sync.dma_start(eo[:], _dchunks(eoT))

                        kv_panel_proj(xo, w_sa, kb_sa, vb_sa, PSK, PV)
                        kg_sa = ag(kb_sa, [NPANEL * D, PANEL])
                        vg_sa = ag(vb_sa, [NPANEL * D, VW // 2])

                        # SA Q projection from prefetched weights (overlaps
                        # the SA collectives)
                        def evq(mc, ps):
                            nc.vector.tensor_copy(QT_sa[:, mc, :], ps[:])
                        proj_from_sbuf(wqs, xo, evq, PSK)

                        kv_panel_proj(eo, {"k": (wck,), "v": w_ca["v"]},
                                      kb_ca, vb_ca, PSK, PV)
                        kg_ca = ag(kb_ca, [NPANEL * D, PANEL])
                        vg_ca = ag(vb_ca, [NPANEL * D, VW // 2])
                    tapit("QTsa", QT_sa)

                    # ---- SA attention ----
                    KT_sa = A_SA.tile([128, NPANEL, DC, PANEL], FP8)
                    v1_sa = A_SA.tile([128, NPANEL, NSC, H, DK + 1], FP8)
                    load_panels(KT_sa, v1_sa, kg_sa, vg_sa, nc.scalar,
                                masked=True)
                    # CA panels load during SA attention (gpsimd queue,
                    # naturally ordered after the CA collectives)
                    KT_ca = A_CA.tile([128, NPANEL, DC, PANEL], FP8)
                    v1_ca = A_CA.tile([128, NPANEL, NSC, H, DK + 1], FP8)
                    load_panels(KT_ca, v1_ca, kg_ca, vg_ca, nc.gpsimd,
                                masked=False)

                    ON_sa = A_SA.tile([128, DC, QS], BF16)
                    attention_inner(QT_sa, KT_sa, v1_sa, mt, ON_sa, masked=True)
                    tapit("ONsa", ON_sa)
                    x1 = attn_epilogue(w_sa["o"], ON_sa, xo, g1, b1, "sa")

                # ---- CA attention ----
                QT_ca = A_CA.tile([128, DC, QS], BF16)
                with tc.tile_pool(name="wq", bufs=6) as WPOOL, \
                     tc.tile_pool(name="psq", bufs=2, space="PSUM") as PSQ:
                    def evq2(mc, ps):
                        nc.vector.tensor_copy(QT_ca[:, mc, :], ps[:])
                    proj_from_dram(w_ca["q"], x1, evq2, PSQ)
                tapit("QTca", QT_ca)
                ON_ca = A_CA.tile([128, DC, QS], BF16)
                attention_inner(QT_ca, KT_ca, v1_ca, None, ON_ca, masked=False)
                tapit("ONca", ON_ca)
                x2 = attn_epilogue(w_ca["o"], ON_ca, x1, g2, b2, "ca")

            # ---- FFN ----
            with tc.tile_pool(name="ffn", bufs=1) as F, \
                 tc.tile_pool(name="tmp2", bufs=2) as TMP, \
                 tc.tile_pool(name="psf", bufs=2, space="PSUM") as PSF, \
                 tc.tile_pool(name="wf", bufs=6) as WPOOL:
                h1 = F.tile([128, FC, QS], BF16)

                def ev1(fc, ps):
                    nc.scalar.activation(h1[:, fc, :], ps[:], AF.Relu,
                                         bias=fb1t[:, fc:fc + 1])
                proj_from_dram(w_ff1, x2, ev1, PSF, n_mc=FC, n_kc=DC)

                tapit("h1a", h1[:, 0:8, :])
                xpre = F.tile([128, DC, QS], F32R)

                def ev2(mc, ps):
                    nc.vector.scalar_tensor_tensor(
                        out=xpre[:, mc, :], in0=ps[:],
                        scalar=fb2t[:, mc:mc + 1], in1=x2[:, mc, :],
                        op0=ALU.add, op1=ALU.add)
                proj_from_dram(w_ff2, h1, ev2, PSF, n_mc=DC, n_kc=FC)

                tapit("xpreff", xpre)
                out = F.tile([128, DC, QS], F32)
                layernorm(xpre, g3, b3, out, TMP, PSF)
                tapit("outf", out)
                tc.strict_bb_all_engine_barrier()
                for mc in range(DC):
                    nc.sync.dma_start(outT[:, mc, :], out[:, mc, :])
            if tap:
                base = 0
                tap_layout.clear()
                for name, t in tapped:
                    sh = t.shape
                    nparts = sh[0]
                    assert len(sh) == 3 and sh[2] == QS
                    tap_layout[name] = (base, sh[1], nparts)
                    for cci in range(sh[1]):
                        nc.sync.dma_start(
                            dbg[0:nparts, base + cci, :].bitcast(t.dtype),
                            t[:, cci, :])
                    base += sh[1]
                assert base <= 40

    nc.compile()
    return nc


_NC_CACHE = None


def _get_nc():
    global _NC_CACHE
    if _NC_CACHE is None:
        _NC_CACHE = _build()
    return _NC_CACHE


def _pack_w(w):
    """[K, M] fp32 -> packed bf16 [M//128, K//512, 128, 512] (see _build)."""
    K, M = w.shape
    nk4, nmc = K // 512, M // 128
    p = w.reshape(nk4, 4, 128, nmc, 128).transpose(3, 0, 2, 1, 4)
    return np.ascontiguousarray(p.reshape(nmc, nk4, 128, 512)
                                .astype(ml_dtypes.bfloat16))


def _prep_in_maps(x, enc, tgt_mask, src_mask,
                  sa_wq, sa_wk, sa_wv, sa_wo,
                  ca_wq, ca_wk, ca_wv, ca_wo,
                  ff_w1, ff_b1, ff_w2, ff_b2,
                  ln1_g, ln1_b, ln2_g, ln2_b, ln3_g, ln3_b):
    f32 = np.float32
    bf16 = ml_dtypes.bfloat16

    def c(a):
        return np.ascontiguousarray(np.asarray(a), dtype=f32)

    def cb(a):
        return np.ascontiguousarray(np.asarray(a, dtype=f32).astype(bf16))

    xTb = [np.asarray(x, dtype=f32)[b].T.astype(bf16) for b in range(B)]
    eTb = [np.asarray(enc, dtype=f32)[b].T.astype(bf16) for b in range(B)]
    tm = np.asarray(tgt_mask)[0, 0].astype(f32).T            # [k, q]
    sm = np.asarray(src_mask)[0, 0, 0].astype(f32)           # [k]
    vms = c(sm.reshape(S // 128, 128).T)                     # [128, 16]

    def percol(v, nchunks):
        return c(np.asarray(v).reshape(nchunks, 128).T)

    lnb = c(np.concatenate(
        [percol(v, DC) for v in [ln1_g, ln1_b, ln2_g, ln2_b, ln3_g, ln3_b]],
        axis=1))
    fb1 = percol(ff_b1, FC)
    fb2 = percol(ff_b2, DC)
    shared = {
        "vms": vms, "lnb": lnb, "fb1": fb1, "fb2": fb2,
        "w_saq": _pack_w(c(sa_wq)), "w_sak": _pack_w(c(sa_wk)),
        "w_sao": _pack_w(c(sa_wo)), "w_sav": cb(sa_wv),
        "w_caq": _pack_w(c(ca_wq)), "w_cak": _pack_w(c(ca_wk)),
        "w_cao": _pack_w(c(ca_wo)), "w_cav": cb(ca_wv),
        "w_ff1": _pack_w(c(ff_w1)), "w_ff2": _pack_w(c(ff_w2)),
    }
    in_maps = []
    for core in range(NCORES):
        b, qi = core // 4, core % 4
        q0 = qi * QS
        m = dict(shared)
        m["xoT"] = np.ascontiguousarray(xTb[b][:, q0:q0 + QS])
        m["eoT"] = np.ascontiguousarray(eTb[b][:, q0:q0 + QS])
        m["msk"] = np.ascontiguousarray(
            tm[:, q0:q0 + QS].reshape(S // 128, 128, QS).astype(bf16))
        in_maps.append(m)
    return in_maps


def _gather_out(res):
    out = np.empty((B, S, D), dtype=np.float32)
    for core in range(NCORES):
        b, qi = core // 4, core % 4
        q0 = qi * QS
        arr = res.results[core]["outT"]  # [128, DC, QS]
        out[b, q0:q0 + QS, :] = arr.transpose(1, 0, 2).reshape(D, QS).T
    return out


def kernel(**inputs):
    in_maps = _prep_in_maps(**inputs)
    nc = _get_nc()
    res = run_bass_kernel_spmd(nc, in_maps, core_ids=list(range(NCORES)))
    return _gather_out(res)


def _profiled_run(inputs):
    """Test-only: run with NTFF tracing to get HW exec time."""
    in_maps = _prep_in_maps(**inputs)
    nc = _get_nc()
    return run_bass_kernel_spmd(nc, in_maps, core_ids=list(range(NCORES)),
                                trace=True)
